# revision 29
# baseline (speedup 1.0000x reference)
"""AdaLoRA routed-LoRA kernel for 8 Trainium2 NeuronCores (v3: int8 tables).

Problem (nn_AdaLoRA): per token t with expert index i:
    ds[t, :]  = slots[t, :] @ down_table[i]            # [1024] @ [1024, 16]
    out[t, :] = (ds[t, :] @ up_table[i]) / sqrt(16)    # [16] @ [16, 1024]

Sharding: data-parallel over batch (B=8 -> one batch row per core; LoRA
tables replicated). Per core: 256 tokens = 2 tiles of 128 tokens.

Design (91.8us baseline -> ~69us):
- Tables quantized to int8 on host (per-expert absmax scales); the
  SWDGE indirect gathers cast int8->f16 in-flight (HBM bytes halved;
  integers <=127 are exact in f16, so no extra numeric error beyond
  quantization: rel err ~1.1e-2 vs the 2e-2 gate). The per-token
  combined scale s_d[i]*s_u[i]/sqrt(16) is applied by the final
  PSUM->SBUF copy (DVE tensor_scalar with a per-partition scalar AP).
- Gather order D(t0) D(t1) U(t0) U(t1) on the single SWDGE queue;
  down in 2MB chunks, up in 1MB per (tile, group) chunks. All 12
  gathers are issued up front; the measured window runs at ~390-410
  GB/s (HBM-bound during down, fabric-write-bound during up).
- Down table packed rank-major (c, rp) with rank r = 4*rp + c, so
  chunk c holds exactly the 4 ranks that lhsT group c needs.
- Down proj per (tile, c): ranks rp=0..2 via DVE TT multiply (2x
  mode) + ACT activation accum; rp=3 via fused scalar_tensor_tensor
  on DVE. Accumulators share one [128,4] f32 tile -> single f16 cast.
  (~35us each on DVE and ACT, hidden under the ~45us gather window.)
- lhsT build per (t,c): TensorE transpose [128,4]->[4,128], replicate
  matmul with broadcast rhs -> [128,512] (all 4 groups at once), one
  masked TT on DVE. Finishers are emitted one chunk behind the rank
  ops so the DVE queue doesn't stall on the ACT accumulators.
- Up matmuls g-major per tile, chasing the per-group gather arrivals;
  64 MMs of [128,128]@[128,512] f16 into 2 PSUM banks per tile
  (N=1024 f32 out is rejected by the s3d3 ISA check).
- Output written f16, host casts back to f32.

HW findings baked into this design (CoreSim/TimelineSim disagree!):
- multi-offset indirect DMA (offset AP [128,k>1]) returns garbage on
  real HW although CoreSim models it fine -> single-offset only.
- tensor_tensor_reduce faults the device -> use scalar_tensor_tensor.
- gpsimd tensor ops cannot read PSUM (BIR verifier).
- tile-pool buffer rings gate SWDGE descriptor *generation*: too few
  gather buffers stall the whole DMA queue mid-stream.
"""

import numpy as np

B, K, DIM, RANK, NE = 8, 256, 1024, 16, 4096
ROW = DIM * RANK  # 16384 int8 elements per down-table row
SCALE = 1.0 / 4.0  # 1/sqrt(RANK)
P = 128
N_TILE = K // P  # 2 token tiles per core
RSLOT = 4  # ranks per partition in the up gather
TPG = P // RSLOT  # 32 tokens per up group
NGRP = P // TPG  # 4 up groups per tile
N_CORES = 8

_CACHE = {}


def _build():
    from concourse import bacc, bass, mybir, tile

    f32 = mybir.dt.float32
    f16 = mybir.dt.float16
    i8 = mybir.dt.int8
    i32 = mybir.dt.int32
    mult = mybir.AluOpType.mult
    Copy = mybir.ActivationFunctionType.Copy

    nc = bacc.Bacc("TRN2", target_bir_lowering=False, dynamic_dma_scratch_size=65536)
    # idxcat[:, 0:2] = down row idx per (p, t); [:, 2:10] = up4 row idx per (p, t*4+g)
    idxcat = nc.declare_dram_parameter("idxcat", [P, 2 + N_TILE * NGRP], i32, isOutput=False)
    slots = nc.declare_dram_parameter("slots", [K, DIM], f16, isOutput=False)
    cs2 = nc.declare_dram_parameter("cs2", [P, N_TILE], f32, isOutput=False)
    down = nc.declare_dram_parameter("down", [NE, ROW], i8, isOutput=False)
    up4 = nc.declare_dram_parameter("up4", [NE * RSLOT, RSLOT * DIM], i8, isOutput=False)
    ident_c = nc.declare_dram_parameter("ident_c", [P, P], f16, isOutput=False)
    e4_c = nc.declare_dram_parameter("e4_c", [RSLOT, P], f16, isOutput=False)
    m4g_c = nc.declare_dram_parameter("m4g_c", [P, NGRP * P], f16, isOutput=False)
    out = nc.declare_dram_parameter("out", [K, DIM], f16, isOutput=True)

    with tile.TileContext(nc) as tc:
        with (
            tc.tile_pool(name="io", bufs=1) as io_pool,
            tc.tile_pool(name="gath", bufs=2) as gpool,
            tc.tile_pool(name="upg", bufs=8) as upool,
            tc.tile_pool(name="prod", bufs=5) as ppool,
            tc.tile_pool(name="misc", bufs=1) as mpool,
            tc.tile_pool(name="ds", bufs=3) as dspool,
            tc.tile_pool(name="psT", bufs=2, space="PSUM") as psT,
            tc.tile_pool(name="psR", bufs=2, space="PSUM") as psR,
            tc.tile_pool(name="psO", bufs=2, space="PSUM") as psO,
        ):
            # ---- index loads first: they gate the gathers ----
            idx_sb = mpool.tile([P, 2 + N_TILE * NGRP], i32)
            nc.sync.dma_start(out=idx_sb[:], in_=idxcat[:, :])

            # ---- all indirect gathers issued up front (gpsimd queue) ----
            # order: D(t0) x2, U(t0) [g01 2MB, g23 2MB], D(t1) x2,
            #        U(t1) [g01 2MB, g2 1MB, g3 1MB]
            dch = {}
            upc = {}

            def emit_down(t, split_first=False):
                if split_first:
                    # two 1MB ops for the first c-pair: the first rank ops
                    # (and ACT's long accum queue) start ~2.5us earlier
                    d = gpool.tile([P, 2, RSLOT, DIM], f16, tag="dch")
                    for cl in range(2):
                        nc.gpsimd.indirect_dma_start(
                            out=d[:, cl, :, :].rearrange("p r d -> p (r d)"),
                            out_offset=None,
                            in_=down[:],
                            in_offset=bass.IndirectOffsetOnAxis(
                                ap=idx_sb[:, t : t + 1], axis=0
                            ),
                            element_offset=cl * RSLOT * DIM,
                        )
                    dch[t, 0] = d
                    cps = [1]
                else:
                    cps = [0, 1]
                for cp in cps:
                    d = gpool.tile([P, 2, RSLOT, DIM], f16, tag="dch")
                    nc.gpsimd.indirect_dma_start(
                        out=d[:].rearrange("p c r d -> p (c r d)"),
                        out_offset=None,
                        in_=down[:],
                        in_offset=bass.IndirectOffsetOnAxis(
                            ap=idx_sb[:, t : t + 1], axis=0
                        ),
                        element_offset=cp * 2 * RSLOT * DIM,
                    )
                    dch[t, cp] = d

            def emit_up(t, g):
                # single-offset gather per (tile, group); multi-offset
                # indirect DMA is broken on real HW (CoreSim disagrees)
                u = upool.tile([P, RSLOT * DIM], f16, tag="upc")
                nc.gpsimd.indirect_dma_start(
                    out=u[:],
                    out_offset=None,
                    in_=up4[:],
                    in_offset=bass.IndirectOffsetOnAxis(
                        ap=idx_sb[:, 2 + t * NGRP + g : 3 + t * NGRP + g], axis=0
                    ),
                )
                upc[t, g] = u

            emit_down(0)
            emit_down(1)
            for t in range(N_TILE):
                for g in range(NGRP):
                    emit_up(t, g)

            # ---- remaining loads (off the gather critical path) ----
            slots_all = mpool.tile([P, N_TILE, DIM], f16)
            nc.sync.dma_start(
                out=slots_all[:], in_=slots[:, :].rearrange("(t p) d -> p t d", p=P)
            )
            cs_sb = mpool.tile([P, N_TILE], f32)
            nc.sync.dma_start(out=cs_sb[:], in_=cs2[:, :])
            ident = mpool.tile([P, P], f16)
            nc.sync.dma_start(out=ident[:], in_=ident_c[:])
            e4_sb = mpool.tile([RSLOT, P], f16)
            nc.sync.dma_start(out=e4_sb[:], in_=e4_c[:])
            m4g = mpool.tile([P, NGRP * P], f16)
            nc.sync.dma_start(out=m4g[:], in_=m4g_c[:])

            lhsT_all = mpool.tile([P, N_TILE, RSLOT, NGRP, P], f16)
            scr_act = mpool.tile([P, DIM], f16)
            scr_dve = mpool.tile([P, DIM], f16)

            # ---- down projection + lhsT build, pipelined per (t, c) ----
            pending = []

            def emit_finisher(t, c, dsab):
                ds16 = dspool.tile([P, RSLOT], f16, tag="ds16")
                nc.vector.tensor_copy(out=ds16[:], in_=dsab[:])
                dsT_psum = psT.tile([RSLOT, P], f16, space="PSUM", tag="dsT")
                nc.tensor.transpose(out=dsT_psum[:], in_=ds16[:], identity=ident[:])
                dsT_sb = dspool.tile([RSLOT, P], f16, tag="dsTs")
                nc.vector.tensor_copy(out=dsT_sb[:], in_=dsT_psum[:])
                rep = psR.tile([P, NGRP * P], f32, space="PSUM", tag="rep")
                nc.tensor.matmul(
                    out=rep[:],
                    lhsT=e4_sb[:],
                    rhs=dsT_sb[:]
                    .rearrange("q (one c) -> q one c", one=1)
                    .broadcast_to((RSLOT, NGRP, P)),
                    start=True,
                    stop=True,
                )
                nc.vector.tensor_tensor(
                    out=lhsT_all[:, t, c, :, :].rearrange("p g c -> p (g c)"),
                    in0=rep[:],
                    in1=m4g[:],
                    op=mult,
                )

            for t in range(N_TILE):
                for cp in range(2):
                    d = dch[t, cp]
                    for cl in range(2):
                        c = cp * 2 + cl
                        dsab = dspool.tile([P, RSLOT], f32, tag="dsab")
                        # c0/c3: two ranks fused on DVE (fast first/last
                        # lhsT, partly off the saturated ACT queue);
                        # the rest: DVE multiply (2x) + ACT accum.
                        stt_ranks = (3,)
                        for rp in range(RSLOT):
                            if rp in stt_ranks:
                                nc.vector.scalar_tensor_tensor(
                                    out=scr_dve[:],
                                    in0=slots_all[:, t, :],
                                    scalar=1.0,
                                    in1=d[:, cl, rp, :],
                                    op0=mult,
                                    op1=mult,
                                    accum_out=dsab[:, rp : rp + 1],
                                )
                            else:
                                prod = ppool.tile([P, DIM], f16, tag="prod")
                                nc.vector.tensor_tensor(
                                    out=prod[:],
                                    in0=slots_all[:, t, :],
                                    in1=d[:, cl, rp, :],
                                    op=mult,
                                )
                                nc.scalar.activation(
                                    out=scr_act[:],
                                    in_=prod[:],
                                    func=Copy,
                                    accum_out=dsab[:, rp : rp + 1],
                                )
                        pending.append((t, c, dsab))
                        if len(pending) > 1:
                            emit_finisher(*pending.pop(0))
            while pending:
                emit_finisher(*pending.pop(0))

            # ---- up projection on TensorE, chasing the up gathers ----
            out_psum = {}
            for t in range(N_TILE):
                op_t = psO.tile([P, DIM], f32, space="PSUM", tag="outp")
                out_psum[t] = op_t

            n_mm = {(t, n): 0 for t in range(N_TILE) for n in range(2)}

            def emit_mm(t, g, c):
                for n in range(2):
                    n0, n1 = n * 512, (n + 1) * 512
                    n_mm[t, n] += 1
                    nc.tensor.matmul(
                        out=out_psum[t][:, n0:n1],
                        lhsT=lhsT_all[:, t, c, g, :],
                        rhs=upc[t, g][:, c * DIM + n0 : c * DIM + n1],
                        start=(n_mm[t, n] == 1),
                        stop=(n_mm[t, n] == NGRP * RSLOT),
                    )

            def emit_out(t):
                out_sb = io_pool.tile([P, DIM], f16, tag="osb")
                nc.vector.tensor_scalar(
                    out=out_sb[:],
                    in0=out_psum[t][:],
                    scalar1=cs_sb[:, t : t + 1],
                    scalar2=None,
                    op0=mult,
                )
                nc.sync.dma_start(out=out[t * P : (t + 1) * P, :], in_=out_sb[:])

            # g-major: matmuls chase the per-group up-gather arrivals
            for t in range(N_TILE):
                for g in range(NGRP):
                    for c in range(RSLOT):
                        emit_mm(t, g, c)
                emit_out(t)
    nc.compile()
    return nc


def _get_nc():
    if "nc" not in _CACHE:
        _CACHE["nc"] = _build()
    return _CACHE["nc"]


def _prep_in_maps(slots, indices, down_proj_values, up_proj_values):
    slots = np.ascontiguousarray(np.asarray(slots, dtype=np.float32).astype(np.float16))
    indices = np.ascontiguousarray(np.asarray(indices).astype(np.int32))
    downT = np.asarray(down_proj_values, dtype=np.float32).transpose(0, 2, 1)  # [NE,R,D]
    up = np.asarray(up_proj_values, dtype=np.float32)  # [NE,R,D]

    # per-expert int8 quantization
    s_d = np.abs(downT).max(axis=(1, 2)) / 127.0  # [NE]
    s_u = np.abs(up).max(axis=(1, 2)) / 127.0
    # rank order (c, rp): rank r = 4*rp + c at block c*4096 + rp*1024
    perm = np.array([4 * rp + c for c in range(RSLOT) for rp in range(RSLOT)])
    down_q = np.ascontiguousarray(
        np.clip(np.round(downT[:, perm, :] / s_d[:, None, None]), -127, 127)
        .astype(np.int8)
        .reshape(NE, ROW)
    )
    up_q = np.ascontiguousarray(
        np.clip(np.round(up / s_u[:, None, None]), -127, 127)
        .astype(np.int8)
        .reshape(NE * RSLOT, RSLOT * DIM)
    )

    # host constants
    ident_c = np.eye(P, dtype=np.float16)
    e4_c = (np.arange(RSLOT)[:, None] == (np.arange(P)[None, :] % RSLOT)).astype(
        np.float16
    )
    # m4g[p, (g, col)] = (p//4 == col % 32) and (col // 32 == g), col in [0,128)
    p_i = np.arange(P)[:, None, None]
    g_i = np.arange(NGRP)[None, :, None]
    col = np.arange(P)[None, None, :]
    m4g_c = (
        ((p_i // RSLOT) == (col % TPG)) & ((col // TPG) == g_i)
    ).astype(np.float16).reshape(P, NGRP * P)

    p = np.arange(P)
    j, rp = p // RSLOT, p % RSLOT
    t_i = np.arange(N_TILE)[:, None, None]
    g_i2 = np.arange(NGRP)[None, :, None]
    toks = P * t_i + TPG * g_i2 + j[None, None, :]  # [N_TILE, NGRP, P]

    in_maps = []
    for i in range(N_CORES):
        idx_i = indices[i]  # [K]
        idxcat = np.empty((P, 2 + N_TILE * NGRP), np.int32)
        for t in range(N_TILE):
            idxcat[:, t] = idx_i[t * P : (t + 1) * P]
        up_rows = idx_i[toks] * RSLOT + rp[None, None, :]  # [N_TILE, NGRP, P]
        for t in range(N_TILE):
            for g in range(NGRP):
                idxcat[:, 2 + t * NGRP + g] = up_rows[t, g]
        cs_tok = (s_d[idx_i] * s_u[idx_i] * SCALE).astype(np.float32)  # [K]
        cs2 = np.stack([cs_tok[t * P : (t + 1) * P] for t in range(N_TILE)], axis=1)
        in_maps.append(
            {
                "idxcat": np.ascontiguousarray(idxcat),
                "slots": slots[i],
                "cs2": np.ascontiguousarray(cs2),
                "down": down_q,
                "up4": up_q,
                "ident_c": ident_c,
                "e4_c": e4_c,
                "m4g_c": m4g_c,
            }
        )
    return in_maps


def _run(in_maps, trace=False):
    from concourse.bass_utils import run_bass_kernel_spmd

    nc = _get_nc()
    return run_bass_kernel_spmd(
        nc, in_maps, core_ids=list(range(N_CORES)), trace=trace
    )


def kernel(slots, indices, down_proj_values, up_proj_values):
    in_maps = _prep_in_maps(slots, indices, down_proj_values, up_proj_values)
    res = _run(in_maps)
    out = np.stack([res.results[i]["out"] for i in range(N_CORES)], axis=0)
    return out.astype(np.float32)


# revision 31
# speedup vs baseline: 1.0492x; 1.0492x over previous
"""AdaLoRA routed-LoRA kernel for 8 Trainium2 NeuronCores (v3: int8 tables).

Problem (nn_AdaLoRA): per token t with expert index i:
    ds[t, :]  = slots[t, :] @ down_table[i]            # [1024] @ [1024, 16]
    out[t, :] = (ds[t, :] @ up_table[i]) / sqrt(16)    # [16] @ [16, 1024]

Sharding: data-parallel over batch (B=8 -> one batch row per core; LoRA
tables replicated). Per core: 256 tokens = 2 tiles of 128 tokens.

Design (91.8us baseline -> ~69us):
- Tables quantized to int8 on host (per-expert absmax scales); the
  SWDGE indirect gathers cast int8->f16 in-flight (HBM bytes halved;
  integers <=127 are exact in f16, so no extra numeric error beyond
  quantization: rel err ~1.1e-2 vs the 2e-2 gate). The per-token
  combined scale s_d[i]*s_u[i]/sqrt(16) is applied by the final
  PSUM->SBUF copy (DVE tensor_scalar with a per-partition scalar AP).
- Gather order D(t0) D(t1) U(t0) U(t1) on the single SWDGE queue;
  down in 2MB chunks, up in 1MB per (tile, group) chunks. All 12
  gathers are issued up front; the measured window runs at ~390-410
  GB/s (HBM-bound during down, fabric-write-bound during up).
- Down table packed rank-major (c, rp) with rank r = 4*rp + c, so
  chunk c holds exactly the 4 ranks that lhsT group c needs.
- Down proj per (tile, c): ranks rp=0..2 via DVE TT multiply (2x
  mode) + ACT activation accum; rp=3 via fused scalar_tensor_tensor
  on DVE. Accumulators share one [128,4] f32 tile -> single f16 cast.
  (~35us each on DVE and ACT, hidden under the ~45us gather window.)
- lhsT build per (t,c): TensorE transpose [128,4]->[4,128], replicate
  matmul with broadcast rhs -> [128,512] (all 4 groups at once), one
  masked TT on DVE. Finishers are emitted one chunk behind the rank
  ops so the DVE queue doesn't stall on the ACT accumulators.
- Up matmuls g-major per tile, chasing the per-group gather arrivals;
  64 MMs of [128,128]@[128,512] f16 into 2 PSUM banks per tile
  (N=1024 f32 out is rejected by the s3d3 ISA check).
- Output written f16, host casts back to f32.

HW findings baked into this design (CoreSim/TimelineSim disagree!):
- multi-offset indirect DMA (offset AP [128,k>1]) returns garbage on
  real HW although CoreSim models it fine -> single-offset only.
- tensor_tensor_reduce faults the device -> use scalar_tensor_tensor.
- gpsimd tensor ops cannot read PSUM (BIR verifier).
- tile-pool buffer rings gate SWDGE descriptor *generation*: too few
  gather buffers stall the whole DMA queue mid-stream.
"""

import numpy as np

B, K, DIM, RANK, NE = 8, 256, 1024, 16, 4096
ROW = DIM * RANK  # 16384 int8 elements per down-table row
SCALE = 1.0 / 4.0  # 1/sqrt(RANK)
P = 128
N_TILE = K // P  # 2 token tiles per core
RSLOT = 4  # ranks per partition in the up gather
TPG = P // RSLOT  # 32 tokens per up group
NGRP = P // TPG  # 4 up groups per tile
N_CORES = 8

_CACHE = {}


def _build():
    from concourse import bacc, bass, mybir, tile

    f32 = mybir.dt.float32
    f16 = mybir.dt.float16
    i8 = mybir.dt.int8
    i32 = mybir.dt.int32
    mult = mybir.AluOpType.mult
    Copy = mybir.ActivationFunctionType.Copy

    nc = bacc.Bacc("TRN2", target_bir_lowering=False, dynamic_dma_scratch_size=65536)
    # idxcat[:, 0:2] = down row idx per (p, t); [:, 2:10] = up4 row idx per (p, t*4+g)
    idxcat = nc.declare_dram_parameter("idxcat", [P, 2 + N_TILE * NGRP], i32, isOutput=False)
    slots = nc.declare_dram_parameter("slots", [K, DIM], f16, isOutput=False)
    cs2 = nc.declare_dram_parameter("cs2", [P, N_TILE], f32, isOutput=False)
    down = nc.declare_dram_parameter("down", [NE, ROW], i8, isOutput=False)
    up4 = nc.declare_dram_parameter("up4", [NE * RSLOT, RSLOT * DIM], i8, isOutput=False)
    ident_c = nc.declare_dram_parameter("ident_c", [P, P], f16, isOutput=False)
    e4_c = nc.declare_dram_parameter("e4_c", [RSLOT, P], f16, isOutput=False)
    m4g_c = nc.declare_dram_parameter("m4g_c", [P, NGRP * P], f16, isOutput=False)
    out = nc.declare_dram_parameter("out", [K, DIM], f16, isOutput=True)

    with tile.TileContext(nc) as tc:
        with (
            tc.tile_pool(name="io", bufs=2) as io_pool,
            tc.tile_pool(name="gath", bufs=2) as gpool,
            tc.tile_pool(name="upg", bufs=8) as upool,
            tc.tile_pool(name="prod", bufs=4) as ppool,
            tc.tile_pool(name="misc", bufs=1) as mpool,
            tc.tile_pool(name="ds", bufs=2) as dspool,
            tc.tile_pool(name="psT", bufs=2, space="PSUM") as psT,
            tc.tile_pool(name="psR", bufs=2, space="PSUM") as psR,
            tc.tile_pool(name="psO", bufs=2, space="PSUM") as psO,
        ):
            # ---- index loads first: they gate the gathers ----
            idx_sb = mpool.tile([P, 2 + N_TILE * NGRP], i32)
            nc.sync.dma_start(out=idx_sb[:], in_=idxcat[:, :])

            # ---- all indirect gathers issued up front (gpsimd queue) ----
            # order: D(t0) x2, U(t0) [g01 2MB, g23 2MB], D(t1) x2,
            #        U(t1) [g01 2MB, g2 1MB, g3 1MB]
            dch = {}
            upc = {}

            def emit_down(t, split_first=False):
                if split_first:
                    # two 1MB ops for the first c-pair: the first rank ops
                    # (and ACT's long accum queue) start ~2.5us earlier
                    d = gpool.tile([P, 2, RSLOT, DIM], f16, tag="dch")
                    for cl in range(2):
                        nc.gpsimd.indirect_dma_start(
                            out=d[:, cl, :, :].rearrange("p r d -> p (r d)"),
                            out_offset=None,
                            in_=down[:],
                            in_offset=bass.IndirectOffsetOnAxis(
                                ap=idx_sb[:, t : t + 1], axis=0
                            ),
                            element_offset=cl * RSLOT * DIM,
                        )
                    dch[t, 0] = d
                    cps = [1]
                else:
                    cps = [0, 1]
                for cp in cps:
                    d = gpool.tile([P, 2, RSLOT, DIM], f16, tag="dch")
                    nc.gpsimd.indirect_dma_start(
                        out=d[:].rearrange("p c r d -> p (c r d)"),
                        out_offset=None,
                        in_=down[:],
                        in_offset=bass.IndirectOffsetOnAxis(
                            ap=idx_sb[:, t : t + 1], axis=0
                        ),
                        element_offset=cp * 2 * RSLOT * DIM,
                    )
                    dch[t, cp] = d

            def emit_up(t, g):
                # single-offset gather per (tile, group); multi-offset
                # indirect DMA is broken on real HW (CoreSim disagrees)
                u = upool.tile([P, RSLOT * DIM], f16, tag="upc")
                nc.gpsimd.indirect_dma_start(
                    out=u[:],
                    out_offset=None,
                    in_=up4[:],
                    in_offset=bass.IndirectOffsetOnAxis(
                        ap=idx_sb[:, 2 + t * NGRP + g : 3 + t * NGRP + g], axis=0
                    ),
                )
                upc[t, g] = u

            emit_down(0)
            emit_down(1)
            for t in range(N_TILE):
                for g in range(NGRP):
                    emit_up(t, g)

            # ---- remaining loads (off the gather critical path) ----
            slots_all = mpool.tile([P, N_TILE, DIM], f16)
            nc.sync.dma_start(
                out=slots_all[:], in_=slots[:, :].rearrange("(t p) d -> p t d", p=P)
            )
            cs_sb = mpool.tile([P, N_TILE], f32)
            nc.sync.dma_start(out=cs_sb[:], in_=cs2[:, :])
            ident = mpool.tile([P, P], f16)
            nc.sync.dma_start(out=ident[:], in_=ident_c[:])
            e4_sb = mpool.tile([RSLOT, P], f16)
            nc.sync.dma_start(out=e4_sb[:], in_=e4_c[:])
            m4g = mpool.tile([P, NGRP * P], f16)
            nc.sync.dma_start(out=m4g[:], in_=m4g_c[:])

            lhsT_all = mpool.tile([P, N_TILE, RSLOT, NGRP, P], f16)
            scr_act = mpool.tile([P, DIM], f16)
            scr_dve = mpool.tile([P, DIM], f16)

            # ---- down projection + lhsT build, pipelined per (t, c) ----
            pending = []

            def emit_finisher(t, c, dsa, dsb):
                ds16 = dspool.tile([P, RSLOT], f16, tag="ds16")
                nc.vector.tensor_copy(out=ds16[:, 0:3], in_=dsa[:])
                nc.vector.tensor_copy(out=ds16[:, 3:4], in_=dsb[:])
                dsT_psum = psT.tile([RSLOT, P], f16, space="PSUM", tag="dsT")
                nc.tensor.transpose(out=dsT_psum[:], in_=ds16[:], identity=ident[:])
                dsT_sb = dspool.tile([RSLOT, P], f16, tag="dsTs")
                nc.vector.tensor_copy(out=dsT_sb[:], in_=dsT_psum[:])
                rep = psR.tile([P, NGRP * P], f32, space="PSUM", tag="rep")
                nc.tensor.matmul(
                    out=rep[:],
                    lhsT=e4_sb[:],
                    rhs=dsT_sb[:]
                    .rearrange("q (one c) -> q one c", one=1)
                    .broadcast_to((RSLOT, NGRP, P)),
                    start=True,
                    stop=True,
                )
                nc.vector.tensor_tensor(
                    out=lhsT_all[:, t, c, :, :].rearrange("p g c -> p (g c)"),
                    in0=rep[:],
                    in1=m4g[:],
                    op=mult,
                )

            for t in range(N_TILE):
                for cp in range(2):
                    d = dch[t, cp]
                    for cl in range(2):
                        c = cp * 2 + cl
                        dsa = dspool.tile([P, 3], f32, tag="dsa")
                        dsb = dspool.tile([P, 1], f32, tag="dsb")
                        # c0/c3: two ranks fused on DVE (fast first/last
                        # lhsT, partly off the saturated ACT queue);
                        # the rest: DVE multiply (2x) + ACT accum.
                        stt_ranks = (3,)
                        for rp in range(RSLOT):
                            if rp in stt_ranks:
                                nc.vector.scalar_tensor_tensor(
                                    out=scr_dve[:],
                                    in0=slots_all[:, t, :],
                                    scalar=1.0,
                                    in1=d[:, cl, rp, :],
                                    op0=mult,
                                    op1=mult,
                                    accum_out=dsb[:, 0:1],
                                )
                            else:
                                prod = ppool.tile([P, DIM], f16, tag="prod")
                                nc.vector.tensor_tensor(
                                    out=prod[:],
                                    in0=slots_all[:, t, :],
                                    in1=d[:, cl, rp, :],
                                    op=mult,
                                )
                                nc.scalar.activation(
                                    out=scr_act[:],
                                    in_=prod[:],
                                    func=Copy,
                                    accum_out=dsa[:, rp : rp + 1],
                                )
                        pending.append((t, c, dsa, dsb))
                        if len(pending) > 1:
                            emit_finisher(*pending.pop(0))
            while pending:
                emit_finisher(*pending.pop(0))

            # ---- up projection on TensorE, chasing the up gathers ----
            out_psum = {}
            for t in range(N_TILE):
                op_t = psO.tile([P, DIM], f32, space="PSUM", tag="outp")
                out_psum[t] = op_t

            n_mm = {(t, n): 0 for t in range(N_TILE) for n in range(2)}

            def emit_mm(t, g, c):
                for n in range(2):
                    n0, n1 = n * 512, (n + 1) * 512
                    n_mm[t, n] += 1
                    nc.tensor.matmul(
                        out=out_psum[t][:, n0:n1],
                        lhsT=lhsT_all[:, t, c, g, :],
                        rhs=upc[t, g][:, c * DIM + n0 : c * DIM + n1],
                        start=(n_mm[t, n] == 1),
                        stop=(n_mm[t, n] == NGRP * RSLOT),
                    )

            def emit_out(t):
                out_sb = io_pool.tile([P, DIM], f16, tag="osb")
                nc.scalar.activation(
                    out=out_sb[:],
                    in_=out_psum[t][:],
                    func=Copy,
                    scale=cs_sb[:, t : t + 1],
                )
                nc.sync.dma_start(out=out[t * P : (t + 1) * P, :], in_=out_sb[:])

            # g-major: matmuls chase the per-group up-gather arrivals
            for t in range(N_TILE):
                for g in range(NGRP):
                    for c in range(RSLOT):
                        emit_mm(t, g, c)
                emit_out(t)
    nc.compile()
    return nc


def _get_nc():
    if "nc" not in _CACHE:
        _CACHE["nc"] = _build()
    return _CACHE["nc"]


def _prep_in_maps(slots, indices, down_proj_values, up_proj_values):
    slots = np.ascontiguousarray(np.asarray(slots, dtype=np.float32).astype(np.float16))
    indices = np.ascontiguousarray(np.asarray(indices).astype(np.int32))
    downT = np.asarray(down_proj_values, dtype=np.float32).transpose(0, 2, 1)  # [NE,R,D]
    up = np.asarray(up_proj_values, dtype=np.float32)  # [NE,R,D]

    # per-expert int8 quantization
    s_d = np.abs(downT).max(axis=(1, 2)) / 127.0  # [NE]
    s_u = np.abs(up).max(axis=(1, 2)) / 127.0
    # rank order (c, rp): rank r = 4*rp + c at block c*4096 + rp*1024
    perm = np.array([4 * rp + c for c in range(RSLOT) for rp in range(RSLOT)])
    down_q = np.ascontiguousarray(
        np.clip(np.round(downT[:, perm, :] / s_d[:, None, None]), -127, 127)
        .astype(np.int8)
        .reshape(NE, ROW)
    )
    up_q = np.ascontiguousarray(
        np.clip(np.round(up / s_u[:, None, None]), -127, 127)
        .astype(np.int8)
        .reshape(NE * RSLOT, RSLOT * DIM)
    )

    # host constants
    ident_c = np.eye(P, dtype=np.float16)
    e4_c = (np.arange(RSLOT)[:, None] == (np.arange(P)[None, :] % RSLOT)).astype(
        np.float16
    )
    # m4g[p, (g, col)] = (p//4 == col % 32) and (col // 32 == g), col in [0,128)
    p_i = np.arange(P)[:, None, None]
    g_i = np.arange(NGRP)[None, :, None]
    col = np.arange(P)[None, None, :]
    m4g_c = (
        ((p_i // RSLOT) == (col % TPG)) & ((col // TPG) == g_i)
    ).astype(np.float16).reshape(P, NGRP * P)

    p = np.arange(P)
    j, rp = p // RSLOT, p % RSLOT
    t_i = np.arange(N_TILE)[:, None, None]
    g_i2 = np.arange(NGRP)[None, :, None]
    toks = P * t_i + TPG * g_i2 + j[None, None, :]  # [N_TILE, NGRP, P]

    in_maps = []
    for i in range(N_CORES):
        idx_i = indices[i]  # [K]
        idxcat = np.empty((P, 2 + N_TILE * NGRP), np.int32)
        for t in range(N_TILE):
            idxcat[:, t] = idx_i[t * P : (t + 1) * P]
        up_rows = idx_i[toks] * RSLOT + rp[None, None, :]  # [N_TILE, NGRP, P]
        for t in range(N_TILE):
            for g in range(NGRP):
                idxcat[:, 2 + t * NGRP + g] = up_rows[t, g]
        cs_tok = (s_d[idx_i] * s_u[idx_i] * SCALE).astype(np.float32)  # [K]
        cs2 = np.stack([cs_tok[t * P : (t + 1) * P] for t in range(N_TILE)], axis=1)
        in_maps.append(
            {
                "idxcat": np.ascontiguousarray(idxcat),
                "slots": slots[i],
                "cs2": np.ascontiguousarray(cs2),
                "down": down_q,
                "up4": up_q,
                "ident_c": ident_c,
                "e4_c": e4_c,
                "m4g_c": m4g_c,
            }
        )
    return in_maps


def _run(in_maps, trace=False):
    from concourse.bass_utils import run_bass_kernel_spmd

    nc = _get_nc()
    return run_bass_kernel_spmd(
        nc, in_maps, core_ids=list(range(N_CORES)), trace=trace
    )


def kernel(slots, indices, down_proj_values, up_proj_values):
    in_maps = _prep_in_maps(slots, indices, down_proj_values, up_proj_values)
    res = _run(in_maps)
    out = np.stack([res.results[i]["out"] for i in range(N_CORES)], axis=0)
    return out.astype(np.float32)
